# revision 30
# baseline (speedup 1.0000x reference)
"""Trainium2 Bass kernel for nn_Attention (channel-attention, 8 NeuronCores).

Algorithm (algebraically identical to the reference):
  The attention contracts over the spatial axis n = 32*32*32 = 32768, and the
  attention matrices are tiny (64x64 per head).  Everything collapses around
  the per-batch Gram matrix G_b = x_b @ x_b^T (128x128):

    scores_bh = scale * Wq_h G_b Wk_h^T            (tiny)
    attn      = softmax(scores)                     (tiny)
    W_eff_b   = (1/n) * sum_h Wout_h attn_bh Wv_h   (64x128, tiny)
    y_b       = W_eff_b @ x_b + b_out               (the only other big matmul)

  Sharding: spatial n split across the 8 cores (4096 each).  Each core
  computes a partial G over its shard (the only big contraction), a 64KB
  bf16 AllReduce combines them, the tiny attention algebra is replicated on
  every core, and each core produces its own n-slice of y.  All matmuls run
  in bf16 with f32 accumulation (rounding errors average out over the huge
  contractions; measured end-to-end max rel err ~1e-4).

  The ncfw entry-barrier collective that Bacc auto-inserts after the
  preamble absorbs the collective-firmware wakeup (~13us doorbell-to-notice
  + ~34us barrier), so the real AllReduce is the SECOND cc op and runs as
  soon as its payload lands.  (A separate warm-up AllReduce only adds ~10us
  of serial ncfw time on top of the barrier -- measured, not speculation.)

  x is shipped once, in [n, c] layout (needed by the Gram matmuls); the
  [c, n] layout needed by the final y matmul is produced on-chip with PE
  transposes scheduled under the AllReduce wait, which also keeps the PE
  HAM-warm through the collective.  y is written back in bf16 (quantization
  ~0.2% <<< the 2e-2 gate) to halve the output-DMA tail.
"""

import numpy as np
import ml_dtypes

import concourse.bass as bass
import concourse.bacc as bacc
import concourse.mybir as mybir
import concourse.tile as tile
from concourse.tile import add_dep_helper
from concourse.bass_utils import run_bass_kernel_spmd

NCORES = 8
P = 128
N_TOT = 32 * 32 * 32          # 32768 spatial points
NSH = N_TOT // NCORES         # 4096 per core per batch
F = 2 * NSH                   # 8192 free columns (both batches side by side)
NCHUNK = 4                    # xn DMA chunks (pipelined with the G matmuls)
DUMMY_WARM_MMS = 62           # HAM warm-keeper fp32 matmuls under the AR wait
HEADS = 8
DH = 64
SCALE = DH ** -0.5
BF = mybir.dt.bfloat16
F32 = mybir.dt.float32
bf16 = ml_dtypes.bfloat16

# kept for test.py's --sim routing patches
RID = 0
NC_PHYS = (0, 1, 2, 3, 6, 7, 4, 5)

_CACHED_NC = None


class _TrimmedTileContext(tile.TileContext):
    """TileContext minus the FINAL all-engine barrier of the exit sequence.

    The stock exit is drain -> barrier -> sem-clear -> barrier; the last
    barrier only makes every engine wait for the gpsimd sem-clear before
    halting, which matters for looped NEFFs but not a single-shot kernel:
    the clear still completes before its issuing engine halts, so a
    re-execution starts with zeroed semaphores either way.  Dropping it
    saves ~4us of measured EVSEM-butterfly tail.
    """

    def _drain_and_barrier(self, tick_clock, wait_clock):
        from concourse.vector_clock import ScopedClock

        drain_inst = self.nc.sync.drain()
        wait_clock.add_sem_waits(
            drain_inst.ins, ScopedClock({None: tick_clock.global_clock})
        )
        self.nc.all_engine_barrier()
        popped = self.nc._tile_sem_poison_stack.pop()
        assert popped is self._sem_poison
        self.nc.clear_and_free_semaphores(list(self.sems.allocated().values()))


def build_nc():
    nc = bacc.Bacc(
        "TRN2", target_bir_lowering=False, debug=False, num_devices=NCORES
    )

    xn_ext = nc.dram_tensor("xn", [P, F], BF, kind="ExternalInput")
    wq_ext = nc.dram_tensor("wqT", [P, 512], BF, kind="ExternalInput")
    wk_ext = nc.dram_tensor("wkT", [P, 512], BF, kind="ExternalInput")
    wv_ext = nc.dram_tensor("wv", [P, 512], BF, kind="ExternalInput")
    wo_ext = nc.dram_tensor("woT", [P, 256], BF, kind="ExternalInput")
    bo_ext = nc.dram_tensor("bout", [P, 1], F32, kind="ExternalInput")
    id_ext = nc.dram_tensor("ident", [P, P], BF, kind="ExternalInput")
    out_ext = nc.dram_tensor("out", [P, NSH], BF, kind="ExternalOutput")

    with _TrimmedTileContext(nc) as tc:
        with (
            tc.tile_pool(name="const", bufs=1) as const,
            tc.tile_pool(name="data", bufs=1) as data,
            tc.tile_pool(name="work", bufs=1) as work,
            tc.tile_pool(name="ypool", bufs=1) as ypool,
            tc.tile_pool(name="psg", bufs=2, space="PSUM") as psg,
            tc.tile_pool(name="psd", bufs=2, space="PSUM") as psd,
            tc.tile_pool(name="psd1", bufs=1, space="PSUM") as psd1,
            tc.tile_pool(name="psy", bufs=2, space="PSUM") as psy,
            tc.tile_pool(name="dram", bufs=1, space="DRAM") as dram,
        ):
            # ---- phase B: xn split across BOTH HWDGE rings; weights queue
            # behind the xn chunks (they are needed only much later) ----
            CH = F // NCHUNK  # 2048 columns (16 n-blocks) per chunk
            xn_tiles = []
            for c in range(NCHUNK):
                t = data.tile([P, CH], BF, tag=f"xn{c}")
                eng = nc.sync if c % 2 == 0 else nc.scalar
                eng.dma_start(t[:], xn_ext[:, c * CH : (c + 1) * CH])
                xn_tiles.append(t)

            wq = const.tile([P, 512], BF, tag="wq")
            nc.sync.dma_start(wq[:], wq_ext[:])
            wk = const.tile([P, 512], BF, tag="wk")
            nc.scalar.dma_start(wk[:], wk_ext[:])
            wv = const.tile([P, 512], BF, tag="wv")
            nc.sync.dma_start(wv[:], wv_ext[:])
            wo = const.tile([P, 256], BF, tag="wo")
            nc.scalar.dma_start(wo[:], wo_ext[:])
            bo = const.tile([P, 1], F32, tag="bo")
            nc.sync.dma_start(bo[:], bo_ext[:])
            ident = const.tile([P, P], BF, tag="ident")
            nc.scalar.dma_start(ident[:], id_ext[:])
            # warm-keeper source, zeroed early while the DVE is idle
            dummy_src = work.tile([P, 512], F32, tag="dummy")
            nc.vector.memset(dummy_src[:], 0.0)

            g_ps = [psg.tile([P, P], F32, tag="g", name=f"g_ps{b}") for b in range(2)]
            for c in range(NCHUNK):
                b = c // 2
                for tl in range(16):
                    gt = (c % 2) * 16 + tl  # accumulation index within batch
                    blk = xn_tiles[c][:, tl * P : (tl + 1) * P]
                    nc.tensor.matmul(
                        g_ps[b][:], blk, blk, start=(gt == 0), stop=(gt == 31)
                    )

            # bf16 partials -> 64KB AllReduce payload
            g_sb = work.tile([P, 256], BF, tag="gsb")
            for b in range(2):
                nc.vector.tensor_copy(g_sb[:, b * P : (b + 1) * P], g_ps[b][:])

            # ---- phase C: AllReduce the Gram over the 8 cores ----
            g_in = dram.tile([P, 256], BF, tag="gin")
            g_out = dram.tile([P, 256], BF, tag="gout", addr_space="Shared")
            g_dma = nc.sync.dma_start(g_in[:], g_sb[:])
            nc.gpsimd.collective_compute(
                "AllReduce",
                mybir.AluOpType.add,
                ins=[g_in.opt()],
                outs=[g_out.opt()],
                replica_groups=[list(range(NCORES))],
            )
            gbf = [
                work.tile([P, P], BF, tag=f"gbf{b}", name=f"gbf{b}")
                for b in range(2)
            ]
            for b in range(2):
                eng = nc.sync if b == 0 else nc.scalar
                eng.dma_start(gbf[b][:], g_out[:, b * P : (b + 1) * P])

            # ---- transpose xn -> xc in [c, n] layout, under the AR wait ----
            # Ordering-only deps on the G-path DMA keep the scheduler from
            # hoisting these ahead of the G matmuls (which would delay the
            # collective trigger); no runtime semaphore is added.
            xc = data.tile([P, F], BF, tag="xc")
            for c in range(NCHUNK):
                for tl in range(16):
                    col = c * CH + tl * P
                    tp = psy.tile([P, P], BF, tag="y", name=f"tp{c}_{tl}")
                    tri = nc.tensor.transpose(
                        tp[:], xn_tiles[c][:, tl * P : (tl + 1) * P], ident[:]
                    )
                    add_dep_helper(
                        tri.ins, g_dma.ins, sync=True,
                        reason="transposes ordered after the G path",
                    )
                    nc.vector.tensor_copy(xc[:, col : col + P], tp[:])

            # Dummy PE work to keep the HAM clock-gate warm through the
            # AllReduce wait so phases D/E run at 2.4 GHz, sized to roughly
            # the expected collective window.  fp32 matmuls run at 4
            # cycles/row (~850ns each), so few instructions cover a long
            # window.  Results are never read; the psum slots are the ones
            # the G partials released.
            last_warm = None
            for w in range(DUMMY_WARM_MMS):
                scratch = psg.tile([P, 512], F32, tag="g", name=f"warm{w}")
                wi = nc.tensor.matmul(
                    scratch[:], dummy_src[:, :P], dummy_src[:],
                    start=True, stop=True,
                )
                add_dep_helper(
                    wi.ins, g_dma.ins, sync=True,
                    reason="warm-keeper ordered after the G path",
                )
                last_warm = wi

            # ---- phase D: scores -> softmax -> W_eff (replicated, tiny) ----
            # scale folded into wqT on the host; 1/n folded into wv.  Both
            # batches packed side by side in every tile so each softmax
            # stage is ONE wide engine op instead of two.
            sums = work.tile([P, 8], F32, tag="sums")
            recip = work.tile([P, 8], F32, tag="recip")
            weff = work.tile([P, 128], BF, tag="weff")
            a_ps = [psd.tile([P, 512], F32, tag="d", name=f"a_ps{b}") for b in range(2)]
            a_sb = [work.tile([P, 512], BF, tag=f"asb{b}", name=f"a_sb{b}") for b in range(2)]
            # s_ps/mt_ps/w_ps have disjoint lifetimes -> one rotating bank
            s_ps = psd1.tile([P, 512], F32, tag="s", name="s_ps")
            negmax = work.tile([P, 8], F32, tag="negmax")
            sm_sb = work.tile([P, 512], F32, tag="smsb")
            exp_sb = work.tile([P, 512], F32, tag="expsb")
            attn = work.tile([P, 512], BF, tag="attn")
            mt_ps = psd1.tile([P, 512], F32, tag="s", name="mt_ps")
            mt_sb = work.tile([P, 512], BF, tag="mtsb")
            w_ps = psd1.tile([P, 128], F32, tag="s", name="w_ps")

            for b in range(2):
                ai = nc.tensor.matmul(
                    a_ps[b][:], gbf[b][:], wq[:],
                    start=True, stop=True,
                )
                if last_warm is not None:
                    add_dep_helper(
                        ai.ins, last_warm.ins, sync=False,
                        reason="phase D after the warm-keeper block",
                    )
            for b in range(2):
                # sliced so the first S matmuls start after slice 0 lands;
                # alternating DVE/ACT so the casts aren't serialized
                for sl in range(4):
                    src = a_ps[b][:, sl * 128 : (sl + 1) * 128]
                    dst = a_sb[b][:, sl * 128 : (sl + 1) * 128]
                    if sl % 2 == 0:
                        nc.vector.tensor_copy(dst, src)
                    else:
                        nc.scalar.activation(
                            dst, src,
                            mybir.ActivationFunctionType.Copy,
                            bias=0.0, scale=1.0,
                        )
            # S[i-half, j-group]: batch b at cols b*256; head h at partitions
            # 64*(h%2), col-group 64*(h//2) within the batch's 256 columns.
            for b in range(2):
                for h in range(HEADS):
                    pb = 64 * (h % 2)
                    cg = b * 256 + 64 * (h // 2)
                    nc.tensor.matmul(
                        s_ps[pb : pb + 64, cg : cg + 64],
                        a_sb[b][:, h * 64 : (h + 1) * 64],
                        wk[:, h * 64 : (h + 1) * 64],
                        start=True, stop=True,
                    )
            # Per-group max subtracted on DVE (cheap, parallel engine) so the
            # exp is ONE wide ACT op instead of 16 serialized ones.
            nc.vector.reduce_max(
                negmax[:],
                s_ps[:].rearrange("p (g j) -> p g j", j=64),
                axis=mybir.AxisListType.X,
                negate=True,
            )
            nc.vector.tensor_tensor(
                sm_sb[:].rearrange("p (g j) -> p g j", j=64),
                s_ps[:].rearrange("p (g j) -> p g j", j=64),
                negmax[:].rearrange("p g -> p g ()").broadcast_to((P, 8, 64)),
                op=mybir.AluOpType.add,
            )
            nc.scalar.activation(
                exp_sb[:],
                sm_sb[:],
                mybir.ActivationFunctionType.Exp,
                bias=0.0,
                scale=1.0,
            )
            nc.vector.reduce_sum(
                sums[:],
                exp_sb[:].rearrange("p (g j) -> p g j", j=64),
                axis=mybir.AxisListType.X,
            )
            nc.vector.reciprocal(recip[:], sums[:])
            nc.vector.tensor_tensor(
                attn[:].rearrange("p (g j) -> p g j", j=64),
                exp_sb[:].rearrange("p (g j) -> p g j", j=64),
                recip[:].rearrange("p g -> p g ()").broadcast_to((P, 8, 64)),
                op=mybir.AluOpType.mult,
            )
            # MT_bh = attn_bh^T @ WoutT_h, same packing as attn/woT
            for b in range(2):
                for h in range(HEADS):
                    pb = 64 * (h % 2)
                    cg = 64 * (h // 2)
                    nc.tensor.matmul(
                        mt_ps[pb : pb + 64, b * 256 + cg : b * 256 + cg + 64],
                        attn[pb : pb + 64, b * 256 + cg : b * 256 + cg + 64],
                        wo[pb : pb + 64, cg : cg + 64],
                        start=True, stop=True,
                    )
            nc.vector.tensor_copy(mt_sb[:], mt_ps[:])
            # W_effT_b[c, o] accumulated over the 4 head-pair chunks
            for b in range(2):
                for g in range(4):
                    nc.tensor.matmul(
                        w_ps[:, b * 64 : (b + 1) * 64],
                        wv[:, g * P : (g + 1) * P],
                        mt_sb[:, b * 256 + g * 64 : b * 256 + (g + 1) * 64],
                        start=(g == 0), stop=(g == 3),
                    )
            nc.vector.tensor_copy(weff[:], w_ps[:])

            # ---- phase E: y = W_eff @ x + b_out, chunked + streamed out ----
            for j in range(8):
                y_ps = psy.tile([P, 512], F32, tag="y", name=f"y_ps{j}")
                for b in range(2):
                    nc.tensor.matmul(
                        y_ps[b * 64 : (b + 1) * 64, :],
                        weff[:, b * 64 : (b + 1) * 64],
                        xc[:, b * NSH + j * 512 : b * NSH + (j + 1) * 512],
                        start=True, stop=True,
                    )
                y_sb = ypool.tile([P, 512], BF, tag=f"y{j}", name=f"y_sb{j}")
                nc.any.tensor_scalar_add(y_sb[:], y_ps[:], bo[:, 0:1])
                if j < 7:
                    eng = nc.sync if j % 2 == 0 else nc.scalar
                    eng.dma_start(out_ext[:, j * 512 : (j + 1) * 512], y_sb[:])
                else:
                    # split the final chunk across both rings to shorten the
                    # tail (its DMA is the last data movement in the kernel)
                    nc.sync.dma_start(
                        out_ext[:, j * 512 : j * 512 + 256], y_sb[:, 0:256]
                    )
                    nc.scalar.dma_start(
                        out_ext[:, j * 512 + 256 : (j + 1) * 512], y_sb[:, 256:512]
                    )

    nc.compile()
    return nc


def _get_nc():
    global _CACHED_NC
    if _CACHED_NC is None:
        _CACHED_NC = build_nc()
    return _CACHED_NC


def make_in_maps(x, w_qkv, w_out, b_out):
    x = np.ascontiguousarray(x, dtype=np.float32)
    w_qkv = np.asarray(w_qkv, dtype=np.float32)
    w_out = np.asarray(w_out, dtype=np.float32)
    b_out = np.asarray(b_out, dtype=np.float32)
    xf = x.reshape(2, P, N_TOT)

    wq_h = np.ascontiguousarray((w_qkv[:512].T * SCALE)).astype(bf16)
    wk_h = np.ascontiguousarray(w_qkv[512:1024].T).astype(bf16)
    wv_h = np.ascontiguousarray(
        (w_qkv[1024:] / N_TOT).reshape(4, P, P).transpose(1, 0, 2).reshape(P, 512)
    ).astype(bf16)
    wo_f = np.zeros((P, 256), np.float32)
    for h in range(HEADS):
        wo_f[
            64 * (h % 2) : 64 * (h % 2) + 64, 64 * (h // 2) : 64 * (h // 2) + 64
        ] = w_out[:, h * 64 : (h + 1) * 64].T
    wo_h = wo_f.astype(bf16)
    bo_h = np.concatenate([b_out, b_out]).reshape(P, 1).astype(np.float32)
    id_h = np.eye(P, dtype=np.float32).astype(bf16)

    in_maps = []
    for c in range(NCORES):
        sh = xf[:, :, c * NSH : (c + 1) * NSH]  # (2, 128, 4096)
        xn_h = np.ascontiguousarray(
            sh.transpose(0, 2, 1)
            .reshape(2, 32, P, P)
            .transpose(2, 0, 1, 3)
            .reshape(P, F)
        ).astype(bf16)
        in_maps.append(
            {
                "xn": xn_h,
                "wqT": wq_h,
                "wkT": wk_h,
                "wv": wv_h,
                "woT": wo_h,
                "bout": bo_h,
                "ident": id_h,
            }
        )
    return in_maps


def assemble_output(results):
    y = np.empty((2, 64, N_TOT), np.float32)
    for c in range(NCORES):
        o = np.asarray(results[c]["out"]).astype(np.float32)  # [128, 4096] bf16
        y[0, :, c * NSH : (c + 1) * NSH] = o[:64]
        y[1, :, c * NSH : (c + 1) * NSH] = o[64:]
    return y.reshape(2, 64, 32, 32, 32)


def kernel(**inputs):
    in_maps = make_in_maps(
        inputs["x"], inputs["w_qkv"], inputs["w_out"], inputs["b_out"]
    )
    nc = _get_nc()
    res = run_bass_kernel_spmd(nc, in_maps, core_ids=list(range(NCORES)))
    return assemble_output(res.results)
